# revision 9
# baseline (speedup 1.0000x reference)
"""Trainium2 Bass kernel for nn_AttentionLayer (B=8, N=1024, D=256, H=4).

Sharding: pure data-parallel over batch B across 8 NeuronCores (one batch
element per core, all parameters replicated). No collectives.

Key observation: the reference multiplies the final output by the query mask,
so rows with mask=0 produce zero output; and the attention keys/denominator
only involve mask=1 rows. Queries and keys therefore share ONE compacted row
set: the host gathers the unmasked rows (max 547 over the 8 batches) into
xc [KPAD=640, D] and the whole layer runs on 5 token chunks. All matmul FREE
dims run at NF = max unmasked count rounded up to 4 (548), not 640 — padded
queries beyond NF are never computed. The host scatters the kernel's [NF, D]
output back into zeros([N, D]).

Host-side precomputes (all exact): every weight is pre-permuted into its
exact SBUF tile layout and pre-cast to bf16, so each weight DMA is one large
contiguous descriptor per partition (the f32 + rearrange path was ~12K 1KB
descriptors and gated the first q/k matmul); wvg = [0.5*wv | wg] concatenated
so v and the gate share one matmul stream; wo_pre = lnr_gamma-folded out_w
with an extra column holding its row-sums (the final-LN mean then falls out
of the projection matmul for free); bias_ext = out_b + lnr_beta @ out_w with
the bias-mean in the extra column. x is sent both f32 (LN stats / residual)
and bf16 (combine), mask as f32+bf16, so no on-device casts remain.

Per-core algorithm (bf16 matmuls, fp32 stats/output, all free-dim layouts):
  xn = LN(xc)                                  (bn_stats; gamma/beta folded
                                                into the transpose copies;
                                                row means kept for the tail)
  xnT via tensor-engine transposes
  per head h:
    q^T, k^T = [e, n] via 512/36 n-splits      (weights stay natural layout)
    s^T  = kT-chunk.T @ qT                     ([key-chunk, n] logits in PSUM)
    esT  = exp(s^T/16)                         (ACT; already the av lhsT layout)
    [v|g] = xn @ wvg                           (one 512-wide stream per chunk)
    out  = esT.T @ [v*mk | mk]                 ([n, 257]; col 256 = denominator)
    t_h  = out * (tanh+1) / denom + xc         (Vector)
    z_h  = lnr-normalize(t_h)                  (emitted per head on GpSimd so
                                                the tail only owns head 3's)
  head 3's av loop interleaves, per token chunk: PE transposes of z (replacing
  the descriptor-flood DMA transposes), the wo_pre projection matmuls, and the
  final-LN tail, all engine-pinned (Vector: combine+stats, GpSimd: f1/out,
  Scalar: rsqrt/square/copies) so the tail paces at the PE rate.
  y    = zT.T @ wo_pre + bias + xc             (col 256 = row-sum -> mean)
  out  = LN_lno(y)
"""

import os
import sys

for _p in ("/opt/trn_rl_repo", "/root/.axon_site/_ro/trn_rl_repo"):
    if os.path.isdir(_p) and _p not in sys.path:
        sys.path.insert(0, _p)
        break

import ml_dtypes
import numpy as np

N, D, H = 1024, 256, 4
FCH = D * H // 128  # 8 feature chunks of z
EPS = 1e-6
SCALE = 1.0 / 16.0
NWARM = 40

BF16 = ml_dtypes.bfloat16

_PROGRAMS = {}  # NF -> built Bass program


def _build_program(NF):
    from contextlib import ExitStack

    import concourse.bass as bass
    import concourse.mybir as mybir
    import concourse.tile as tile
    from concourse import bacc
    from concourse.masks import make_identity

    KCH = (NF + 127) // 128  # token chunks
    KPAD = 128 * KCH
    f32 = mybir.dt.float32
    bf16 = mybir.dt.bfloat16
    AF = mybir.ActivationFunctionType
    OP = mybir.AluOpType

    nc = bacc.Bacc(
        "TRN2",
        target_bir_lowering=False,
        debug=False,
        enable_asserts=False,
        num_devices=8,
    )

    x_d = nc.dram_tensor("x", [KPAD, D], f32, kind="ExternalInput")
    xbf_d = nc.dram_tensor("x_bf", [128, KCH, D], bf16, kind="ExternalInput")
    smf_d = nc.dram_tensor("sm_f", [128, KCH + 4], f32, kind="ExternalInput")
    smb_d = nc.dram_tensor("sm_b", [128, KCH], bf16, kind="ExternalInput")
    wq_d = nc.dram_tensor("wq", [128, H, 2, D], bf16, kind="ExternalInput")
    wk_d = nc.dram_tensor("wk", [128, H, 2, D], bf16, kind="ExternalInput")
    wvg_d = nc.dram_tensor("wvg", [128, H, 2, 2 * D], bf16, kind="ExternalInput")
    wo_d = nc.dram_tensor("wo_pre", [128, H, 2, D + 1], bf16, kind="ExternalInput")
    be_d = nc.dram_tensor("bias_ext", [D + 1], f32, kind="ExternalInput")
    lnog_d = nc.dram_tensor("lno_g", [D], f32, kind="ExternalInput")
    lnob_d = nc.dram_tensor("lno_b", [D], f32, kind="ExternalInput")
    y_d = nc.dram_tensor("y", [KPAD, D], f32, kind="ExternalOutput")

    def bcast_ap(ap, parts=128):
        return bass.AP(
            tensor=ap.tensor, offset=ap.offset, ap=[[0, parts]] + list(ap.ap)
        )

    with tile.TileContext(nc) as tc, ExitStack() as ctx:
        const = ctx.enter_context(tc.tile_pool(name="const", bufs=1))
        big = ctx.enter_context(tc.tile_pool(name="big", bufs=1))
        hpool = ctx.enter_context(tc.tile_pool(name="hpool", bufs=2))
        spool = ctx.enter_context(tc.tile_pool(name="spool", bufs=11))
        small = ctx.enter_context(tc.tile_pool(name="small", bufs=3))
        ps_s = ctx.enter_context(tc.tile_pool(name="ps_s", bufs=2, space="PSUM"))
        ps_o = ctx.enter_context(tc.tile_pool(name="ps_o", bufs=2, space="PSUM"))
        ps_vg = ctx.enter_context(tc.tile_pool(name="ps_vg", bufs=2, space="PSUM"))

        # ---- stage 0: weights first (they gate the first q/k matmul),
        # x chunks on the sync ring so LN starts on chunk 0 early
        wq_bf = const.tile([128, H, 2, D], bf16)
        wk_bf = const.tile([128, H, 2, D], bf16)
        wvg_bf = const.tile([128, H, 2, 2 * D], bf16)
        nc.gpsimd.dma_start(out=wq_bf, in_=wq_d.ap())
        nc.gpsimd.dma_start(out=wk_bf, in_=wk_d.ap())
        nc.scalar.dma_start(out=wvg_bf, in_=wvg_d.ap())
        sm_f = const.tile([128, KCH + 4], f32)
        nc.scalar.dma_start(out=sm_f, in_=smf_d.ap())
        mk_bf = const.tile([128, KCH], bf16)
        nc.scalar.dma_start(out=mk_bf, in_=smb_d.ap())

        x_sb = const.tile([128, KCH, D + 1], f32)  # col D = per-row mean
        for c in range(KCH):
            nc.sync.dma_start(
                out=x_sb[:, c, 0:D], in_=x_d.ap()[128 * c : 128 * (c + 1), :]
            )
        x_bf = const.tile([128, KCH, D], bf16)
        nc.sync.dma_start(out=x_bf, in_=xbf_d.ap())

        mk_f = sm_f[:, 0:KCH]

        ident = const.tile([128, 128], bf16)
        make_identity(nc, ident)

        eps_t = const.tile([128, 1], f32)
        nc.vector.memset(eps_t, EPS)
        zero_t = const.tile([128, 1], f32)
        nc.vector.memset(zero_t, 0.0)
        # touch Rsqrt so its ACT table loads during the x-DMA wait instead
        # of inside the first layernorm's critical chain
        rs_warm = const.tile([128, 1], f32)
        nc.scalar.activation(
            out=rs_warm, in_=eps_t, func=AF.Sqrt, bias=eps_t[:], scale=1.0
        )

        # dummy matmuls to trip the PE HAM clock-gate before the real
        # stream begins (PE would otherwise sit cold through the LN ramp)
        warm_sink = const.tile([128, 128], f32)
        warm_ps = ps_o.tile([128, D + 1], f32, tag="o")
        for i in range(NWARM):
            nc.tensor.matmul(
                warm_ps[:, 0:128], lhsT=ident, rhs=ident,
                start=(i == 0), stop=(i == NWARM - 1),
            )
        nc.any.tensor_copy(out=warm_sink, in_=warm_ps[:, 0:128])

        # ---- stage 1: layernorm + xnT (per-chunk pipelined, transposes on
        # the tensor engine: no DMA-xbar mode switches)
        xn = big.tile([128, KCH, D], bf16)
        xnT = const.tile([128, 2, KPAD], bf16)  # [p, dc, n] = xn^T[128*dc+p, n]
        for c in range(KCH):
            st6 = small.tile([128, 6], f32, tag="st6")
            nc.vector.bn_stats(out=st6, in_=x_sb[:, c, 0:D])
            mv = small.tile([128, 2], f32, tag="mv")
            nc.vector.bn_aggr(out=mv, in_=st6)
            nc.any.tensor_copy(out=x_sb[:, c, D : D + 1], in_=mv[:, 0:1])
            rs = small.tile([128, 1], f32, tag="rs")
            nc.scalar.activation(
                out=rs, in_=mv[:, 1:2], func=AF.Sqrt, bias=eps_t[:], scale=1.0
            )
            nc.vector.reciprocal(rs, rs)
            nc.vector.tensor_scalar(
                xn[:, c, :], x_sb[:, c, 0:D], mv[:, 0:1], rs, OP.subtract, OP.mult
            )
            for dc in range(2):
                tr_ps = ps_vg.tile([128, 512], bf16, tag="pvg")
                nc.tensor.transpose(
                    tr_ps[:, 0:128], xn[:, c, 128 * dc : 128 * dc + 128], ident
                )
                # gamma/beta land here: after the transpose d is the
                # partition dim, so they are plain per-partition scalars
                nc.any.tensor_scalar(
                    xnT[:, dc, 128 * c : 128 * c + 128],
                    tr_ps[:, 0:128],
                    sm_f[:, KCH + dc : KCH + dc + 1],
                    sm_f[:, KCH + 2 + dc : KCH + 3 + dc],
                    OP.mult,
                    OP.add,
                )

        # ---- stage 2: heads
        t_all = big.tile([128, H, KCH, D], bf16, tag="tz")
        mv_r = big.tile([128, H, KCH, 2], f32)
        z = big.tile([128, KCH, D * H], bf16)  # [p(n), c, h*256+e]
        zT = big.tile([128, FCH, KPAD], bf16)  # [p, fc, n] = z^T[128*fc+p, n]
        y_sb = big.tile([128, KCH, D], bf16)
        y_out = big.tile([128, KCH, D], f32)

        NSPLITS = ((0, 512), (512, NF - 512)) if NF > 512 else ((0, NF),)

        def scopy(out, in_):
            nc.scalar.activation(
                out=out, in_=in_, func=AF.Copy, bias=0.0, scale=1.0
            )

        def transpose_z(c):
            # PE transposes of z chunk c into zT (replaces the DMA-xbar
            # transpose whose descriptor flood dominated the old tail)
            cw = min(128, NF - 128 * c)
            copy_eng = (scopy, scopy, nc.vector.tensor_copy,
                        nc.vector.tensor_copy, scopy, scopy,
                        nc.vector.tensor_copy, nc.vector.tensor_copy)
            for half in range(2):
                tr_ps = ps_vg.tile([128, 512], bf16, tag="pvg")
                for j in range(4):
                    k = 4 * half + j
                    nc.tensor.transpose(
                        tr_ps[:, 128 * j : 128 * j + cw],
                        z[0:cw, c, 128 * k : 128 * k + 128],
                        ident[0:cw, 0:cw],
                    )
                for j in range(4):
                    k = 4 * half + j
                    copy_eng[k](
                        out=zT[:, k, 128 * c : 128 * c + cw],
                        in_=tr_ps[:, 128 * j : 128 * j + cw],
                    )

        def y_chunk(c):
            # final projection + residual + lno for one token chunk;
            # mean comes from wo_pre's row-sum column + the x row means
            cw = min(128, NF - 128 * c)
            y_ps = ps_s.tile([128, D + 1], f32, tag="s")
            out_ps = y_ps if cw == 128 else y_ps[0:cw, :]
            for kc in range(FCH):
                nc.tensor.matmul(
                    out_ps,
                    lhsT=zT[:, kc, 128 * c : 128 * c + cw],
                    rhs=wo_bf[:, kc // 2, kc % 2, :],
                    start=(kc == 0),
                    stop=(kc == FCH - 1),
                )
            nc.vector.tensor_add(
                y_sb[0:cw, c, :], y_ps[0:cw, 0:D], xb[0:cw, c, 0:D]
            )
            mu = small.tile([128, 1], f32, tag="mu")
            nc.vector.scalar_tensor_tensor(
                out=mu[0:cw, :], in0=y_ps[0:cw, D : D + 1], scalar=1.0 / D,
                in1=xb[0:cw, c, D : D + 1], op0=OP.mult, op1=OP.add,
            )
            musq = small.tile([128, 1], f32, tag="musq")
            nc.vector.tensor_mul(musq[0:cw, :], mu[0:cw, :], mu[0:cw, :])
            sq_scr = small.tile([128, D], bf16, tag="sq")
            s2 = small.tile([128, 1], f32, tag="s2")
            nc.scalar.activation(
                out=sq_scr[0:cw, :], in_=y_sb[0:cw, c, :], func=AF.Square,
                bias=zero_t[0:cw, :], scale=1.0, accum_out=s2[0:cw, :],
            )
            var = small.tile([128, 1], f32, tag="var")
            nc.vector.scalar_tensor_tensor(
                out=var[0:cw, :], in0=s2[0:cw, :], scalar=1.0 / D,
                in1=musq[0:cw, :], op0=OP.mult, op1=OP.subtract,
            )
            rso = small.tile([128, 1], f32, tag="rs")
            nc.scalar.activation(
                out=rso[0:cw, :], in_=var[0:cw, :], func=AF.Sqrt,
                bias=eps_t[0:cw, :], scale=1.0
            )
            nc.vector.reciprocal(rso[0:cw, :], rso[0:cw, :])
            f1 = small.tile([128, D], bf16, tag="f1")
            nc.vector.scalar_tensor_tensor(
                out=f1[0:cw, :], in0=y_sb[0:cw, c, :], scalar=mu[0:cw, :],
                in1=lnog16_bc[0:cw, :], op0=OP.subtract, op1=OP.mult,
            )
            nc.vector.scalar_tensor_tensor(
                out=y_out[0:cw, c, :], in0=f1[0:cw, :], scalar=rso[0:cw, :],
                in1=lnob_bc[0:cw, :], op0=OP.mult, op1=OP.add,
            )
            eng = nc.sync if c % 2 == 0 else nc.scalar
            eng.dma_start(
                out=y_d.ap()[128 * c : 128 * c + cw, :], in_=y_out[0:cw, c, :]
            )

        for h in range(H):
            # q^T, k^T = [e, n] projections (weights stay natural: no
            # weight transposes needed)
            qT_bf = hpool.tile([128, 2, KPAD], bf16, tag="qT")
            kT_bf = hpool.tile([128, 2, KPAD], bf16, tag="kT")
            for wsrc, wdst in ((wq_bf, qT_bf), (wk_bf, kT_bf)):
                for ec in range(2):
                    for m0, mw in NSPLITS:
                        p_ps = ps_vg.tile([128, 512], f32, tag="pvg")
                        for kd in range(2):
                            nc.tensor.matmul(
                                p_ps[:, 0:mw],
                                lhsT=wsrc[:, h, kd, 128 * ec : 128 * ec + 128],
                                rhs=xnT[:, kd, m0 : m0 + mw],
                                start=(kd == 0),
                                stop=(kd == 1),
                            )
                        nc.any.tensor_copy(
                            out=wdst[:, ec, m0 : m0 + mw], in_=p_ps[:, 0:mw]
                        )

            # [v | gate] in one 512-wide stream per chunk
            # v2 = [v * mk | mk] (0.5 pre-folded into wv on host)
            v2 = hpool.tile([128, KCH, D + 2], bf16, tag="v2")
            tanh_o = hpool.tile([128, KCH, D], bf16, tag="tanh")
            for mc in range(KCH):
                pm = min(128, NF - 128 * mc)
                vg_ps = ps_vg.tile([128, 512], f32, tag="pvg")
                for kd in range(2):
                    nc.tensor.matmul(
                        vg_ps[0:pm, :],
                        lhsT=xnT[:, kd, 128 * mc : 128 * mc + pm],
                        rhs=wvg_bf[:, h, kd, :],
                        start=(kd == 0),
                        stop=(kd == 1),
                    )
                nc.any.tensor_scalar(
                    v2[0:pm, mc, 0:D], vg_ps[0:pm, 0:D],
                    mk_f[0:pm, mc : mc + 1], None, OP.mult
                )
                nc.scalar.activation(
                    out=tanh_o[0:pm, mc, :], in_=vg_ps[0:pm, D : 2 * D],
                    func=AF.Tanh, bias=zero_t[0:pm, :], scale=0.5,
                )
            nc.any.tensor_copy(out=v2[:, :, D], in_=mk_bf)

            # logits transposed: s^T tiles [m-chunk, n]; exp output is the
            # av lhsT layout directly (no transpose); 512/36 n-split is
            # forced by the PSUM bank boundary
            esT_tiles = []
            for mc in range(KCH):
                pm = min(128, NF - 128 * mc)
                s_ps = ps_s.tile([128, NF], f32, tag="s")
                for m0, mw in NSPLITS:
                    for kc in range(2):
                        nc.tensor.matmul(
                            s_ps[0:pm, m0 : m0 + mw],
                            lhsT=kT_bf[:, kc, 128 * mc : 128 * mc + pm],
                            rhs=qT_bf[:, kc, m0 : m0 + mw],
                            start=(kc == 0),
                            stop=(kc == 1),
                        )
                esT = spool.tile([128, NF], bf16, tag="esT")
                nc.scalar.activation(
                    out=esT[0:pm, :], in_=s_ps[0:pm, :], func=AF.Exp,
                    bias=zero_t[0:pm, :], scale=SCALE
                )
                esT_tiles.append(esT)

            for c in range(KCH):
                cw = min(128, NF - 128 * c)
                o_ps = ps_o.tile([128, D + 1], f32, tag="o")
                for mc in range(KCH):
                    pm = min(128, NF - 128 * mc)
                    nc.tensor.matmul(
                        o_ps[0:cw, :],
                        lhsT=esT_tiles[mc][0:pm, 128 * c : 128 * c + cw],
                        rhs=v2[0:pm, mc, 0 : D + 1],
                        start=(mc == 0),
                        stop=(mc == KCH - 1),
                    )
                # interleave the tail of earlier chunks into head 3's av
                # stream so projection matmuls and LN hide under av work
                if h == H - 1 and c >= 1:
                    transpose_z(c - 1)
                if h == H - 1 and c >= 2:
                    y_chunk(c - 2)
                hf = small.tile([128, 1], f32, tag="hf")
                nc.vector.reciprocal(hf[0:cw, :], o_ps[0:cw, D : D + 1])
                tmp = small.tile([128, D], bf16, tag="tmp")
                nc.vector.scalar_tensor_tensor(
                    out=tmp[0:cw, :],
                    in0=tanh_o[0:cw, c, :],
                    scalar=1.0,
                    in1=o_ps[0:cw, 0:D],
                    op0=OP.add,
                    op1=OP.mult,
                )
                nc.vector.scalar_tensor_tensor(
                    out=t_all[0:cw, h, c, :],
                    in0=tmp[0:cw, :],
                    scalar=hf[0:cw, :],
                    in1=x_bf[0:cw, c, :],
                    op0=OP.mult,
                    op1=OP.add,
                )
                st6 = small.tile([128, 6], f32, tag="st6")
                nc.vector.bn_stats(out=st6[0:cw, :], in_=t_all[0:cw, h, c, :])
                nc.vector.bn_aggr(out=mv_r[0:cw, h, c, :], in_=st6[0:cw, :])
                # lnr-normalize into z right away: per-head emission keeps
                # all but head 3's z-writes out of the tail
                rsh = small.tile([128, 1], f32, tag="rsh")
                nc.scalar.activation(
                    out=rsh[0:cw, :], in_=mv_r[0:cw, h, c, 1:2], func=AF.Sqrt,
                    bias=eps_t[0:cw, :], scale=1.0,
                )
                nc.vector.reciprocal(rsh[0:cw, :], rsh[0:cw, :])
                nc.vector.tensor_scalar(
                    z[0:cw, c, D * h : D * (h + 1)],
                    t_all[0:cw, h, c, :],
                    mv_r[0:cw, h, c, 0:1],
                    rsh[0:cw, :],
                    OP.subtract,
                    OP.mult,
                )

            if h == 1:
                # out_w / bias prep emitted mid-kernel: DMAs overlap head
                # compute, results only needed at the tail
                wo_bf = const.tile([128, H, 2, D + 1], bf16)
                nc.gpsimd.dma_start(out=wo_bf, in_=wo_d.ap())
                lnog16_bc = const.tile([128, D], bf16)
                nc.gpsimd.dma_start(out=lnog16_bc, in_=bcast_ap(lnog_d.ap()))
                lnob_bc = const.tile([128, D], f32)
                nc.gpsimd.dma_start(out=lnob_bc, in_=bcast_ap(lnob_d.ap()))
                bias_bc = const.tile([128, D + 1], f32)
                nc.gpsimd.dma_start(out=bias_bc, in_=bcast_ap(be_d.ap()))

            if h == 2:
                # xb = x + bias, col 256 = mean(x row) + mean(bias)
                xb = const.tile([128, KCH, D + 1], f32)
                for c in range(KCH):
                    nc.any.tensor_add(xb[:, c, :], x_sb[:, c, :], bias_bc)
                # touch Square so its table is resident before the first
                # interleaved y_chunk
                sq_warm = small.tile([128, 1], f32, tag="rsh")
                nc.scalar.activation(
                    out=sq_warm, in_=eps_t, func=AF.Square,
                    bias=zero_t[:], scale=1.0,
                )

        transpose_z(KCH - 1)
        y_chunk(KCH - 2)
        y_chunk(KCH - 1)

    nc.compile()
    return nc


def _get_program(NF):
    if NF not in _PROGRAMS:
        _PROGRAMS[NF] = _build_program(NF)
    return _PROGRAMS[NF]


def _make_in_maps(inputs):
    full = {k: np.asarray(v, dtype=np.float32) if np.asarray(v).dtype != np.int32
            else np.asarray(v) for k, v in inputs.items()}
    idxs = [np.nonzero(np.asarray(inputs["mask"][b], dtype=np.int32))[0]
            for b in range(8)]
    max_cnt = max(len(i) for i in idxs)
    NF = max(4, ((max_cnt + 3) // 4) * 4)
    KCH = (NF + 127) // 128
    KPAD = 128 * KCH

    # host-side exact weight precomputes (shared across cores), each
    # pre-permuted into its SBUF tile layout and pre-cast to bf16
    wvg = np.concatenate([0.5 * full["wv"], full["wg"]], axis=2)  # [H, D, 2D]
    gvec = np.repeat(full["lnr_g"], H)  # f = e*H + h -> gamma[e]
    bvec = np.repeat(full["lnr_b"], H)
    wo_g = gvec[:, None] * full["out_w"]  # [D*H, D]
    wo_pre = np.concatenate([wo_g, wo_g.sum(axis=1, keepdims=True)], axis=1)
    bias = full["out_b"] + bvec @ full["out_w"]  # [D]
    bias_ext = np.concatenate([bias, [bias.mean()]]).astype(np.float32)

    def perm_w(w):  # [H, D, L] -> [128, H, 2, L] with w[p,h,c,:] = w[h,128c+p,:]
        L = w.shape[2]
        return np.ascontiguousarray(
            w.reshape(H, 2, 128, L).transpose(2, 0, 1, 3)
        ).astype(BF16)

    # wo: row (128b+p)*H+h -> [p, h, b, col]
    wo_b = np.ascontiguousarray(
        wo_pre.reshape(2, 128, H, D + 1).transpose(1, 2, 0, 3)
    ).astype(BF16)

    shared = {
        "wq": perm_w(full["wq"]),
        "wk": perm_w(full["wk"]),
        "wvg": perm_w(wvg),
        "wo_pre": wo_b,
        "bias_ext": bias_ext,
        "lno_g": full["lno_g"], "lno_b": full["lno_b"],
    }
    lng2 = full["ln_g"].reshape(2, 128).T  # [128, 2]
    lnb2 = full["ln_b"].reshape(2, 128).T
    in_maps = []
    for b in range(8):
        idx = idxs[b]
        idx_pad = np.zeros(KPAD, dtype=np.int64)
        idx_pad[: len(idx)] = idx
        mk = np.zeros(KPAD, dtype=np.float32)
        mk[: len(idx)] = 1.0
        mk_pc = mk.reshape(KCH, 128).T  # [128, KCH]
        xg = full["x"][b][idx_pad]  # [KPAD, D]
        m = dict(shared)
        m["x"] = np.ascontiguousarray(xg)
        m["x_bf"] = np.ascontiguousarray(
            xg.reshape(KCH, 128, D).transpose(1, 0, 2)
        ).astype(BF16)
        m["sm_f"] = np.ascontiguousarray(
            np.concatenate([mk_pc, lng2, lnb2], axis=1)
        ).astype(np.float32)
        m["sm_b"] = np.ascontiguousarray(mk_pc).astype(BF16)
        in_maps.append(m)
    return in_maps, idxs, NF


def _scatter_out(results, idxs):
    out = np.zeros((8, N, D), dtype=np.float32)
    for b in range(8):
        yb = results[b]
        out[b, idxs[b], :] = yb[: len(idxs[b])]
    return out


def run_on_hw(inputs, trace=False):
    """Run on the 8 NeuronCores; returns (output [8,1024,256] f32, results obj)."""
    from concourse import bass_utils

    in_maps, idxs, NF = _make_in_maps(inputs)
    nc = _get_program(NF)
    res = bass_utils.run_bass_kernel_spmd(
        nc, in_maps, core_ids=list(range(8)), trace=trace
    )
    out = _scatter_out([res.results[b]["y"] for b in range(8)], idxs)
    return out, res


def _run_sim(inputs):
    """CoreSim fallback (slow but exact): used only if hardware runs fail."""
    from concourse.bass_interp import CoreSim

    in_maps, idxs, NF = _make_in_maps(inputs)
    nc = _get_program(NF)
    outs = []
    for b in range(8):
        sim = CoreSim(nc, trace=False)
        for name, val in in_maps[b].items():
            sim.tensor(name)[:] = val
        sim.simulate(check_with_hw=False)
        outs.append(sim.tensor("y").copy())
    return _scatter_out(outs, idxs)


def kernel(**inputs) -> np.ndarray:
    last_err = None
    for _ in range(3):
        try:
            out, _ = run_on_hw(inputs, trace=False)
        except Exception as e:  # transient PJRT/compile hiccups: retry
            last_err = e
            continue
        if np.isfinite(out).all():
            return out
    try:
        return _run_sim(inputs)
    except Exception:
        if last_err is not None:
            raise last_err
        raise


# revision 11
# speedup vs baseline: 1.3167x; 1.3167x over previous
"""Trainium2 Bass kernel for nn_AttentionLayer (B=8, N=1024, D=256, H=4).

Sharding: pure data-parallel over batch B across 8 NeuronCores (one batch
element per core, all parameters replicated). No collectives.

Key observation: the reference multiplies the final output by the query mask,
so rows with mask=0 produce zero output; and the attention keys/denominator
only involve mask=1 rows. Queries and keys therefore share ONE compacted row
set: the host gathers the unmasked rows (max 547 over the 8 batches) into
xc [KPAD=640, D] and the whole layer runs on 5 token chunks. All matmul FREE
dims run at NF = max unmasked count rounded up to 4 (548), not 640 — padded
queries beyond NF are never computed. The host scatters the kernel's [NF, D]
output back into zeros([N, D]).

Host-side precomputes (all exact): every weight is pre-permuted into its
exact SBUF tile layout and pre-cast to bf16, so each weight DMA is one large
contiguous descriptor per partition (the f32 + rearrange path was ~12K 1KB
descriptors and gated the first q/k matmul); wvg = [0.5*wv | wg] concatenated
so v and the gate share one matmul stream; wo_pre = lnr_gamma-folded out_w
with an extra column holding its row-sums (the final-LN mean then falls out
of the projection matmul for free); bias_ext = out_b + lnr_beta @ out_w with
the bias-mean in the extra column. x is sent both f32 (LN stats / residual)
and bf16 (combine), mask as f32+bf16, so no on-device casts remain.

Per-core algorithm (bf16 matmuls, fp32 stats/output, all free-dim layouts):
  xn = LN(xc)                                  (bn_stats; gamma/beta folded
                                                into the transpose copies;
                                                row means kept for the tail)
  xnT via tensor-engine transposes
  per head h:
    q^T, k^T = [e, n] via 512/36 n-splits      (weights stay natural layout)
    s^T  = kT-chunk.T @ qT                     ([key-chunk, n] logits in PSUM)
    esT  = exp(s^T/16)                         (ACT; already the av lhsT layout)
    [v|g] = xn @ wvg                           (one 512-wide stream per chunk)
    out  = esT.T @ [v*mk | mk]                 ([n, 257]; col 256 = denominator)
    t_h  = out * (tanh+1) / denom + xc         (Vector)
    z_h  = lnr-normalize(t_h)                  (emitted per head on GpSimd so
                                                the tail only owns head 3's)
  head 3's av loop interleaves, per token chunk: PE transposes of z (replacing
  the descriptor-flood DMA transposes), the wo_pre projection matmuls, and the
  final-LN tail, all engine-pinned (Vector: combine+stats, GpSimd: f1/out,
  Scalar: rsqrt/square/copies) so the tail paces at the PE rate.
  y    = zT.T @ wo_pre + bias + xc             (col 256 = row-sum -> mean)
  out  = LN_lno(y)
"""

import os
import sys

for _p in ("/opt/trn_rl_repo", "/root/.axon_site/_ro/trn_rl_repo"):
    if os.path.isdir(_p) and _p not in sys.path:
        sys.path.insert(0, _p)
        break

import ml_dtypes
import numpy as np

N, D, H = 1024, 256, 4
FCH = D * H // 128  # 8 feature chunks of z
EPS = 1e-6
SCALE = 1.0 / 16.0
NWARM = 56

BF16 = ml_dtypes.bfloat16

_PROGRAMS = {}  # NF -> built Bass program


def _build_program(NF):
    from contextlib import ExitStack

    import concourse.bass as bass
    import concourse.mybir as mybir
    import concourse.tile as tile
    from concourse import bacc
    from concourse.masks import make_identity

    KCH = (NF + 127) // 128  # token chunks
    KPAD = 128 * KCH
    f32 = mybir.dt.float32
    bf16 = mybir.dt.bfloat16
    AF = mybir.ActivationFunctionType
    OP = mybir.AluOpType

    nc = bacc.Bacc(
        "TRN2",
        target_bir_lowering=False,
        debug=False,
        enable_asserts=False,
        num_devices=8,
    )

    xbf_d = nc.dram_tensor("x_bf", [128, KCH, D], bf16, kind="ExternalInput")
    smf_d = nc.dram_tensor("sm_f", [128, KCH + 4], f32, kind="ExternalInput")
    smb_d = nc.dram_tensor("sm_b", [128, KCH], bf16, kind="ExternalInput")
    wq_d = nc.dram_tensor("wq", [128, H, 2, D], bf16, kind="ExternalInput")
    wk_d = nc.dram_tensor("wk", [128, H, 2, D], bf16, kind="ExternalInput")
    wvg_d = nc.dram_tensor("wvg", [128, H, 2, 2 * D], bf16, kind="ExternalInput")
    wo_d = nc.dram_tensor("wo_pre", [128, H, 2, D + 1], bf16, kind="ExternalInput")
    be_d = nc.dram_tensor("bias_ext", [D + 1], f32, kind="ExternalInput")
    lnog_d = nc.dram_tensor("lno_g", [D], f32, kind="ExternalInput")
    lnob_d = nc.dram_tensor("lno_b", [D], f32, kind="ExternalInput")
    y_d = nc.dram_tensor("y", [KPAD, D], f32, kind="ExternalOutput")

    def bcast_ap(ap, parts=128):
        return bass.AP(
            tensor=ap.tensor, offset=ap.offset, ap=[[0, parts]] + list(ap.ap)
        )

    with tile.TileContext(nc) as tc, ExitStack() as ctx:
        const = ctx.enter_context(tc.tile_pool(name="const", bufs=1))
        big = ctx.enter_context(tc.tile_pool(name="big", bufs=1))
        hpool = ctx.enter_context(tc.tile_pool(name="hpool", bufs=2))
        spool = ctx.enter_context(tc.tile_pool(name="spool", bufs=11))
        small = ctx.enter_context(tc.tile_pool(name="small", bufs=3))
        ps_s = ctx.enter_context(tc.tile_pool(name="ps_s", bufs=2, space="PSUM"))
        ps_o = ctx.enter_context(tc.tile_pool(name="ps_o", bufs=2, space="PSUM"))
        ps_vg = ctx.enter_context(tc.tile_pool(name="ps_vg", bufs=2, space="PSUM"))

        # ---- stage 0: weights first (they gate the first q/k matmul),
        # x chunks on the sync ring so LN starts on chunk 0 early
        wq_bf = const.tile([128, H, 2, D], bf16)
        wk_bf = const.tile([128, H, 2, D], bf16)
        wvg_bf = const.tile([128, H, 2, 2 * D], bf16)
        nc.gpsimd.dma_start(out=wq_bf, in_=wq_d.ap())
        nc.gpsimd.dma_start(out=wk_bf, in_=wk_d.ap())
        sm_f = const.tile([128, KCH + 4], f32)
        nc.scalar.dma_start(out=sm_f, in_=smf_d.ap())
        mk_bf = const.tile([128, KCH], bf16)
        nc.scalar.dma_start(out=mk_bf, in_=smb_d.ap())
        nc.scalar.dma_start(out=wvg_bf, in_=wvg_d.ap())

        x_bf = const.tile([128, KCH, D], bf16)
        nc.sync.dma_start(out=x_bf, in_=xbf_d.ap())
        xmean = const.tile([128, KCH], f32)

        mk_f = sm_f[:, 0:KCH]

        ident = const.tile([128, 128], bf16)
        make_identity(nc, ident)

        eps_t = const.tile([128, 1], f32)
        nc.vector.memset(eps_t, EPS)
        zero_t = const.tile([128, 1], f32)
        nc.vector.memset(zero_t, 0.0)
        # touch Rsqrt so its ACT table loads during the x-DMA wait instead
        # of inside the first layernorm's critical chain
        rs_warm = const.tile([128, 1], f32)
        nc.scalar.activation(
            out=rs_warm, in_=eps_t, func=AF.Sqrt, bias=eps_t[:], scale=1.0
        )

        # dummy matmuls to trip the PE HAM clock-gate before the real
        # stream begins (PE would otherwise sit cold through the LN ramp)
        warm_sink = const.tile([128, 128], f32)
        warm_ps = ps_o.tile([128, D + 1], f32, tag="o")
        for i in range(NWARM):
            nc.tensor.matmul(
                warm_ps[:, 0:128], lhsT=ident, rhs=ident,
                start=(i == 0), stop=(i == NWARM - 1),
            )
        nc.any.tensor_copy(out=warm_sink, in_=warm_ps[:, 0:128])

        # ---- stage 1: layernorm + xnT (per-chunk pipelined, transposes on
        # the tensor engine: no DMA-xbar mode switches)
        xn = big.tile([128, KCH, D], bf16)
        xnT = const.tile([128, 2, KPAD], bf16)  # [p, dc, n] = xn^T[128*dc+p, n]
        for c in range(KCH):
            st6 = small.tile([128, 6], f32, tag="st6")
            nc.vector.bn_stats(out=st6, in_=x_bf[:, c, :])
            mv = small.tile([128, 2], f32, tag="mv")
            nc.vector.bn_aggr(out=mv, in_=st6)
            nc.any.tensor_copy(out=xmean[:, c : c + 1], in_=mv[:, 0:1])
            rs = small.tile([128, 1], f32, tag="rs")
            nc.scalar.activation(
                out=rs, in_=mv[:, 1:2], func=AF.Sqrt, bias=eps_t[:], scale=1.0
            )
            nc.vector.reciprocal(rs, rs)
            nc.vector.tensor_scalar(
                xn[:, c, :], x_bf[:, c, :], mv[:, 0:1], rs, OP.subtract, OP.mult
            )
            for dc in range(2):
                tr_ps = ps_vg.tile([128, 512], bf16, tag="pvg")
                nc.tensor.transpose(
                    tr_ps[:, 0:128], xn[:, c, 128 * dc : 128 * dc + 128], ident
                )
                # gamma/beta land here: after the transpose d is the
                # partition dim, so they are plain per-partition scalars
                nc.any.tensor_scalar(
                    xnT[:, dc, 128 * c : 128 * c + 128],
                    tr_ps[:, 0:128],
                    sm_f[:, KCH + dc : KCH + dc + 1],
                    sm_f[:, KCH + 2 + dc : KCH + 3 + dc],
                    OP.mult,
                    OP.add,
                )

        # ---- stage 2: heads
        t_all = big.tile([128, H, KCH, D], bf16, tag="tz")
        mv_r = big.tile([128, H, KCH, 2], f32)
        nc.vector.memset(mv_r, 0.0)  # chunk-4 pad partitions stay readable
        z = big.tile([128, KCH, D * H], bf16)  # [p(n), c, h*256+e]
        zT = big.tile([128, FCH, KPAD], bf16)  # [p, fc, n] = z^T[128*fc+p, n]
        y_sb = big.tile([128, KCH, D], bf16)
        y_out = big.tile([128, KCH, D], f32)

        NSPLITS = ((0, 512), (512, NF - 512)) if NF > 512 else ((0, NF),)

        def scopy(out, in_):
            nc.scalar.activation(
                out=out, in_=in_, func=AF.Copy, bias=0.0, scale=1.0
            )

        def transpose_z(c):
            # PE transposes of z chunk c into zT (replaces the DMA-xbar
            # transpose whose descriptor flood dominated the old tail)
            cw = min(128, NF - 128 * c)
            copy_eng = (scopy, scopy, nc.vector.tensor_copy,
                        nc.vector.tensor_copy, scopy, scopy,
                        nc.vector.tensor_copy, nc.vector.tensor_copy)
            for half in range(2):
                tr_ps = ps_vg.tile([128, 512], bf16, tag="pvg")
                for j in range(4):
                    k = 4 * half + j
                    nc.tensor.transpose(
                        tr_ps[:, 128 * j : 128 * j + cw],
                        z[0:cw, c, 128 * k : 128 * k + 128],
                        ident[0:cw, 0:cw],
                    )
                for j in range(4):
                    k = 4 * half + j
                    copy_eng[k](
                        out=zT[:, k, 128 * c : 128 * c + cw],
                        in_=tr_ps[:, 128 * j : 128 * j + cw],
                    )

        def y_chunk(c):
            # final projection + residual + lno for one token chunk;
            # mean comes from wo_pre's row-sum column + the x row means
            cw = min(128, NF - 128 * c)
            y_ps = ps_s.tile([128, D + 1], f32, tag="s")
            out_ps = y_ps if cw == 128 else y_ps[0:cw, :]
            for kc in range(FCH):
                nc.tensor.matmul(
                    out_ps,
                    lhsT=zT[:, kc, 128 * c : 128 * c + cw],
                    rhs=wo_bf[:, kc // 2, kc % 2, :],
                    start=(kc == 0),
                    stop=(kc == FCH - 1),
                )
            nc.vector.tensor_add(
                y_sb[0:cw, c, :], y_ps[0:cw, 0:D], xb[0:cw, c, 0:D]
            )
            mu = small.tile([128, 1], f32, tag="mu")
            nc.vector.scalar_tensor_tensor(
                out=mu[0:cw, :], in0=y_ps[0:cw, D : D + 1], scalar=1.0 / D,
                in1=xb[0:cw, c, D : D + 1], op0=OP.mult, op1=OP.add,
            )
            musq = small.tile([128, 1], f32, tag="musq")
            nc.vector.tensor_mul(musq[0:cw, :], mu[0:cw, :], mu[0:cw, :])
            sq_scr = small.tile([128, D], bf16, tag="sq")
            s2 = small.tile([128, 1], f32, tag="s2")
            nc.scalar.activation(
                out=sq_scr[0:cw, :], in_=y_sb[0:cw, c, :], func=AF.Square,
                bias=zero_t[0:cw, :], scale=1.0, accum_out=s2[0:cw, :],
            )
            var = small.tile([128, 1], f32, tag="var")
            nc.vector.scalar_tensor_tensor(
                out=var[0:cw, :], in0=s2[0:cw, :], scalar=1.0 / D,
                in1=musq[0:cw, :], op0=OP.mult, op1=OP.subtract,
            )
            rso = small.tile([128, 1], f32, tag="rs")
            nc.scalar.activation(
                out=rso[0:cw, :], in_=var[0:cw, :], func=AF.Sqrt,
                bias=eps_t[0:cw, :], scale=1.0
            )
            nc.vector.reciprocal(rso[0:cw, :], rso[0:cw, :])
            f1 = small.tile([128, D], bf16, tag="f1")
            nc.vector.scalar_tensor_tensor(
                out=f1[0:cw, :], in0=y_sb[0:cw, c, :], scalar=mu[0:cw, :],
                in1=lnog16_bc[0:cw, :], op0=OP.subtract, op1=OP.mult,
            )
            nc.vector.scalar_tensor_tensor(
                out=y_out[0:cw, c, :], in0=f1[0:cw, :], scalar=rso[0:cw, :],
                in1=lnob_bc[0:cw, :], op0=OP.mult, op1=OP.add,
            )
            nc.sync.dma_start(
                out=y_d.ap()[128 * c : 128 * c + cw, :], in_=y_out[0:cw, c, :]
            )

        for h in range(H):
            # q^T, k^T = [e, n] projections (weights stay natural: no
            # weight transposes needed)
            qT_bf = hpool.tile([128, 2, KPAD], bf16, tag="qT")
            kT_bf = hpool.tile([128, 2, KPAD], bf16, tag="kT")
            for wsrc, wdst in ((wq_bf, qT_bf), (wk_bf, kT_bf)):
                for ec in range(2):
                    for m0, mw in NSPLITS:
                        p_ps = ps_vg.tile([128, 512], f32, tag="pvg")
                        for kd in range(2):
                            nc.tensor.matmul(
                                p_ps[:, 0:mw],
                                lhsT=wsrc[:, h, kd, 128 * ec : 128 * ec + 128],
                                rhs=xnT[:, kd, m0 : m0 + mw],
                                start=(kd == 0),
                                stop=(kd == 1),
                            )
                        nc.any.tensor_copy(
                            out=wdst[:, ec, m0 : m0 + mw], in_=p_ps[:, 0:mw]
                        )

            # [v | gate] in one 512-wide stream per chunk
            # v2 = [v * mk | mk] (0.5 pre-folded into wv on host)
            v2 = hpool.tile([128, KCH, D + 2], bf16, tag="v2")
            tanh_o = hpool.tile([128, KCH, D], bf16, tag="tanh")
            for mc in range(KCH):
                pm = min(128, NF - 128 * mc)
                vg_ps = ps_vg.tile([128, 512], f32, tag="pvg")
                for kd in range(2):
                    nc.tensor.matmul(
                        vg_ps[0:pm, :],
                        lhsT=xnT[:, kd, 128 * mc : 128 * mc + pm],
                        rhs=wvg_bf[:, h, kd, :],
                        start=(kd == 0),
                        stop=(kd == 1),
                    )
                nc.any.tensor_scalar(
                    v2[0:pm, mc, 0:D], vg_ps[0:pm, 0:D],
                    mk_f[0:pm, mc : mc + 1], None, OP.mult
                )
                nc.scalar.activation(
                    out=tanh_o[0:pm, mc, :], in_=vg_ps[0:pm, D : 2 * D],
                    func=AF.Tanh, bias=zero_t[0:pm, :], scale=0.5,
                )
            nc.any.tensor_copy(out=v2[:, :, D], in_=mk_bf)

            # logits transposed: s^T tiles [m-chunk, n]; exp output is the
            # av lhsT layout directly (no transpose); 512/36 n-split is
            # forced by the PSUM bank boundary
            esT_tiles = []
            for mc in range(KCH):
                pm = min(128, NF - 128 * mc)
                s_ps = ps_s.tile([128, NF], f32, tag="s")
                for m0, mw in NSPLITS:
                    for kc in range(2):
                        nc.tensor.matmul(
                            s_ps[0:pm, m0 : m0 + mw],
                            lhsT=kT_bf[:, kc, 128 * mc : 128 * mc + pm],
                            rhs=qT_bf[:, kc, m0 : m0 + mw],
                            start=(kc == 0),
                            stop=(kc == 1),
                        )
                esT = spool.tile([128, NF], bf16, tag="esT")
                nc.scalar.activation(
                    out=esT[0:pm, :], in_=s_ps[0:pm, :], func=AF.Exp,
                    bias=zero_t[0:pm, :], scale=SCALE
                )
                esT_tiles.append(esT)

            for c in range(KCH):
                cw = min(128, NF - 128 * c)
                o_ps = ps_o.tile([128, D + 1], f32, tag="o")
                for mc in range(KCH):
                    pm = min(128, NF - 128 * mc)
                    nc.tensor.matmul(
                        o_ps[0:cw, :],
                        lhsT=esT_tiles[mc][0:pm, 128 * c : 128 * c + cw],
                        rhs=v2[0:pm, mc, 0 : D + 1],
                        start=(mc == 0),
                        stop=(mc == KCH - 1),
                    )
                # interleave the tail of earlier chunks into head 3's av
                # stream so projection matmuls and LN hide under av work
                if h == H - 1 and c >= 1:
                    transpose_z(c - 1)
                if h == H - 1 and c >= 2:
                    y_chunk(c - 2)
                hf = small.tile([128, 1], f32, tag="hf")
                nc.vector.reciprocal(hf[0:cw, :], o_ps[0:cw, D : D + 1])
                tmp = small.tile([128, D], bf16, tag="tmp")
                nc.vector.scalar_tensor_tensor(
                    out=tmp[0:cw, :],
                    in0=tanh_o[0:cw, c, :],
                    scalar=1.0,
                    in1=o_ps[0:cw, 0:D],
                    op0=OP.add,
                    op1=OP.mult,
                )
                nc.vector.scalar_tensor_tensor(
                    out=t_all[0:cw, h, c, :],
                    in0=tmp[0:cw, :],
                    scalar=hf[0:cw, :],
                    in1=x_bf[0:cw, c, :],
                    op0=OP.mult,
                    op1=OP.add,
                )
                st6 = small.tile([128, 6], f32, tag="st6")
                nc.vector.bn_stats(out=st6[0:cw, :], in_=t_all[0:cw, h, c, :])
                nc.vector.bn_aggr(out=mv_r[0:cw, h, c, :], in_=st6[0:cw, :])
                if h == H - 1:
                    # head 3's z must be ready per chunk for the transpose
                    # pipeline; earlier heads batch their z after the loop
                    rsh = small.tile([128, 1], f32, tag="rsh")
                    nc.scalar.activation(
                        out=rsh[0:cw, :], in_=mv_r[0:cw, h, c, 1:2],
                        func=AF.Sqrt, bias=eps_t[0:cw, :], scale=1.0,
                    )
                    nc.vector.reciprocal(rsh[0:cw, :], rsh[0:cw, :])
                    nc.vector.tensor_scalar(
                        z[0:cw, c, D * h : D * (h + 1)],
                        t_all[0:cw, h, c, :],
                        mv_r[0:cw, h, c, 0:1],
                        rsh[0:cw, :],
                        OP.subtract,
                        OP.mult,
                    )

            if h < H - 1:
                # lnr-normalize head h's chunks into z in one batch; runs
                # on Vector under head h+1's q/k matmul stream
                rsb = small.tile([128, KCH], f32, tag="rsb")
                nc.scalar.activation(
                    out=rsb, in_=mv_r[:, h, :, 1], func=AF.Sqrt,
                    bias=eps_t[:], scale=1.0,
                )
                nc.vector.reciprocal(rsb, rsb)
                for c in range(KCH):
                    cw = min(128, NF - 128 * c)
                    nc.vector.tensor_scalar(
                        z[0:cw, c, D * h : D * (h + 1)],
                        t_all[0:cw, h, c, :],
                        mv_r[0:cw, h, c, 0:1],
                        rsb[0:cw, c : c + 1],
                        OP.subtract,
                        OP.mult,
                    )

            if h == 1:
                # out_w / bias prep emitted mid-kernel: DMAs overlap head
                # compute, results only needed at the tail
                wo_bf = const.tile([128, H, 2, D + 1], bf16)
                nc.gpsimd.dma_start(out=wo_bf, in_=wo_d.ap())
                lnog16_bc = const.tile([128, D], bf16)
                nc.gpsimd.dma_start(out=lnog16_bc, in_=bcast_ap(lnog_d.ap()))
                lnob_bc = const.tile([128, D], f32)
                nc.gpsimd.dma_start(out=lnob_bc, in_=bcast_ap(lnob_d.ap()))
                bias_bc = const.tile([128, D + 1], f32)
                nc.gpsimd.dma_start(out=bias_bc, in_=bcast_ap(be_d.ap()))

            if h == 2:
                # xb = x + bias, col 256 = mean(x row) + mean(bias)
                xb = const.tile([128, KCH, D + 1], f32)
                for c in range(KCH):
                    nc.any.tensor_add(xb[:, c, 0:D], x_bf[:, c, :], bias_bc[:, 0:D])
                    nc.any.tensor_add(
                        xb[:, c, D : D + 1], xmean[:, c : c + 1],
                        bias_bc[:, D : D + 1],
                    )
                # touch Square so its table is resident before the first
                # interleaved y_chunk
                sq_warm = small.tile([128, 1], f32, tag="rsh")
                nc.scalar.activation(
                    out=sq_warm, in_=eps_t, func=AF.Square,
                    bias=zero_t[:], scale=1.0,
                )

        transpose_z(KCH - 1)
        y_chunk(KCH - 2)
        y_chunk(KCH - 1)

    nc.compile()
    return nc


def _get_program(NF):
    if NF not in _PROGRAMS:
        _PROGRAMS[NF] = _build_program(NF)
    return _PROGRAMS[NF]


def _make_in_maps(inputs):
    full = {k: np.asarray(v, dtype=np.float32) if np.asarray(v).dtype != np.int32
            else np.asarray(v) for k, v in inputs.items()}
    idxs = [np.nonzero(np.asarray(inputs["mask"][b], dtype=np.int32))[0]
            for b in range(8)]
    max_cnt = max(len(i) for i in idxs)
    NF = max(4, ((max_cnt + 3) // 4) * 4)
    KCH = (NF + 127) // 128
    KPAD = 128 * KCH

    # host-side exact weight precomputes (shared across cores), each
    # pre-permuted into its SBUF tile layout and pre-cast to bf16
    wvg = np.concatenate([0.5 * full["wv"], full["wg"]], axis=2)  # [H, D, 2D]
    gvec = np.repeat(full["lnr_g"], H)  # f = e*H + h -> gamma[e]
    bvec = np.repeat(full["lnr_b"], H)
    wo_g = gvec[:, None] * full["out_w"]  # [D*H, D]
    wo_pre = np.concatenate([wo_g, wo_g.sum(axis=1, keepdims=True)], axis=1)
    bias = full["out_b"] + bvec @ full["out_w"]  # [D]
    bias_ext = np.concatenate([bias, [bias.mean()]]).astype(np.float32)

    def perm_w(w):  # [H, D, L] -> [128, H, 2, L] with w[p,h,c,:] = w[h,128c+p,:]
        L = w.shape[2]
        return np.ascontiguousarray(
            w.reshape(H, 2, 128, L).transpose(2, 0, 1, 3)
        ).astype(BF16)

    # wo: row (128b+p)*H+h -> [p, h, b, col]
    wo_b = np.ascontiguousarray(
        wo_pre.reshape(2, 128, H, D + 1).transpose(1, 2, 0, 3)
    ).astype(BF16)

    shared = {
        "wq": perm_w(full["wq"]),
        "wk": perm_w(full["wk"]),
        "wvg": perm_w(wvg),
        "wo_pre": wo_b,
        "bias_ext": bias_ext,
        "lno_g": full["lno_g"], "lno_b": full["lno_b"],
    }
    lng2 = full["ln_g"].reshape(2, 128).T  # [128, 2]
    lnb2 = full["ln_b"].reshape(2, 128).T
    in_maps = []
    for b in range(8):
        idx = idxs[b]
        idx_pad = np.zeros(KPAD, dtype=np.int64)
        idx_pad[: len(idx)] = idx
        mk = np.zeros(KPAD, dtype=np.float32)
        mk[: len(idx)] = 1.0
        mk_pc = mk.reshape(KCH, 128).T  # [128, KCH]
        xg = full["x"][b][idx_pad]  # [KPAD, D]
        m = dict(shared)
        m["x_bf"] = np.ascontiguousarray(
            xg.reshape(KCH, 128, D).transpose(1, 0, 2)
        ).astype(BF16)
        m["sm_f"] = np.ascontiguousarray(
            np.concatenate([mk_pc, lng2, lnb2], axis=1)
        ).astype(np.float32)
        m["sm_b"] = np.ascontiguousarray(mk_pc).astype(BF16)
        in_maps.append(m)
    return in_maps, idxs, NF


def _scatter_out(results, idxs):
    out = np.zeros((8, N, D), dtype=np.float32)
    for b in range(8):
        yb = results[b]
        out[b, idxs[b], :] = yb[: len(idxs[b])]
    return out


def run_on_hw(inputs, trace=False):
    """Run on the 8 NeuronCores; returns (output [8,1024,256] f32, results obj)."""
    from concourse import bass_utils

    in_maps, idxs, NF = _make_in_maps(inputs)
    nc = _get_program(NF)
    res = bass_utils.run_bass_kernel_spmd(
        nc, in_maps, core_ids=list(range(8)), trace=trace
    )
    out = _scatter_out([res.results[b]["y"] for b in range(8)], idxs)
    return out, res


def _run_sim(inputs):
    """CoreSim fallback (slow but exact): used only if hardware runs fail."""
    from concourse.bass_interp import CoreSim

    in_maps, idxs, NF = _make_in_maps(inputs)
    nc = _get_program(NF)
    outs = []
    for b in range(8):
        sim = CoreSim(nc, trace=False)
        for name, val in in_maps[b].items():
            sim.tensor(name)[:] = val
        sim.simulate(check_with_hw=False)
        outs.append(sim.tensor("y").copy())
    return _scatter_out(outs, idxs)


def kernel(**inputs) -> np.ndarray:
    last_err = None
    for _ in range(3):
        try:
            out, _ = run_on_hw(inputs, trace=False)
        except Exception as e:  # transient PJRT/compile hiccups: retry
            last_err = e
            continue
        if np.isfinite(out).all():
            return out
    try:
        return _run_sim(inputs)
    except Exception:
        if last_err is not None:
            raise last_err
        raise
